# revision 1
# baseline (speedup 1.0000x reference)
"""Trainium2 Bass kernel for nn_KNNModule_2946347565933.

Effective computation (batch/KNN collapse to a residual delta-MLP; `batch` is
unused by the reference):
    w = lrelu(bn(weights @ ri_W0)); w = lrelu(bn(w @ ri_W1))
    for l in 0..3:  h = lrelu(bn(w @ dW0[l])); d = h @ dW1[l] + db1[l]
                    pos += d[:, :2]; w += d[:, 2:]
    h = lrelu(bn(w @ ro_W0)); w_out = h @ ro_W1 + ro_b1
    return pos, w_out

Strategy (8 cores, data-parallel over N=400000):
 - channels-on-partitions layout: per-core residual stream [128, 50000] bf16
   resident in SBUF; matmuls keep weights stationary, rows moving (N=500/tile).
 - 7 BN sync points. Layer-1 stats are computed on host (exact, from the 2x2
   second-moment of `weights`). The other 6 use bn_stats per tile + bn_aggr,
   then a tiny AllGather of (count, mean, count*var) records and one more
   bn_aggr to merge across cores.
 - after each sync, the pre-activation is recomputed on the PE (cheaper than
   storing it), normalized+lrelu'd in ONE ScalarE activation
   (Lrelu(s*a+t), per-partition s,t), and the next layer's pre-activation +
   stats are fused into the same pass.
 - linear biases ahead of BN cancel exactly in BN; db1/ro_b1 and the final
   pos accumulation are applied on host (pos never touches the device).
"""
import os
import sys

sys.path.insert(0, "/opt/trn_rl_repo")

from contextlib import ExitStack

import ml_dtypes
import numpy as np

import concourse.bass as bass
import concourse.bacc as bacc
import concourse.mybir as mybir
import concourse.tile as tile
from concourse.bass_utils import run_bass_kernel_spmd

F32 = mybir.dt.float32
BF16 = mybir.dt.float16  # fp16: same PE rate as bf16, 8x finer mantissa

NCORES = 8
N, D, C_IN, H, C_OUT, L = 400000, 2, 2, 128, 2, 4
R = N // NCORES          # rows per core
TF = 500                 # tile free size (rows per tile)
T = R // TF              # tiles per pass
EPS = 1e-5
SLOPE = 0.01

_cache = {}


def _install_trace_hook():
    """Recreate the missing antenv.axon_hooks NTFF-profile hook via ctypes so
    run_bass_kernel_spmd(trace=True) can capture device profiles under axon."""
    import types

    if "antenv.axon_hooks" not in sys.modules:
        mod = types.ModuleType("antenv.axon_hooks")
        mod._h = None
        mod.set_axon_ntff_profile_hook = lambda h: setattr(mod, "_h", h)
        mod.get_axon_ntff_profile_hook = lambda: mod._h
        sys.modules["antenv.axon_hooks"] = mod
        import antenv

        antenv.axon_hooks = mod
    from antenv.axon_hooks import (
        get_axon_ntff_profile_hook,
        set_axon_ntff_profile_hook,
    )

    if get_axon_ntff_profile_hook() is None:
        if "/root/.axon_site" not in sys.path:
            sys.path.insert(0, "/root/.axon_site")
        from trn_agent_boot.trn_boot import _ntff_profile_via_ctypes

        set_axon_ntff_profile_hook(
            _ntff_profile_via_ctypes("/opt/axon/libaxon_pjrt.so"))
    import concourse.bass_utils as bu

    bu.upload_artifacts = lambda tmpdir: "local://" + tmpdir


def _build():
    nc = bacc.Bacc("TRN2", target_bir_lowering=False, debug=False,
                   num_devices=NCORES)
    # ---- I/O ----
    w0t_d = nc.dram_tensor("w0t", [C_IN, R], BF16, kind="ExternalInput")
    riW0_d = nc.dram_tensor("riW0", [C_IN, H], BF16, kind="ExternalInput")
    riW1_d = nc.dram_tensor("riW1", [H, H], BF16, kind="ExternalInput")
    dW0_d = nc.dram_tensor("dW0", [L, H, H], BF16, kind="ExternalInput")
    dW1w_d = nc.dram_tensor("dW1w", [L, H, H], BF16, kind="ExternalInput")
    dW1p_d = nc.dram_tensor("dW1p", [L, H, D], BF16, kind="ExternalInput")
    roW0_d = nc.dram_tensor("roW0", [H, H], BF16, kind="ExternalInput")
    roW1_d = nc.dram_tensor("roW1", [H, C_OUT], BF16, kind="ExternalInput")
    # per-partition BN params: col k = BN layer k+2 (layers 2..7)
    g_d = nc.dram_tensor("gT", [H, 6], F32, kind="ExternalInput")
    be_d = nc.dram_tensor("beT", [H, 6], F32, kind="ExternalInput")
    s1t1_d = nc.dram_tensor("s1t1", [H, 2], F32, kind="ExternalInput")

    dpos_d = nc.dram_tensor("dpos", [L, D, R], BF16, kind="ExternalOutput")
    wout_d = nc.dram_tensor("wout", [C_OUT, R], F32, kind="ExternalOutput")

    with tile.TileContext(nc) as tc, ExitStack() as ctx:
        P = H
        sb = ctx.enter_context(tc.tile_pool(name="sb", bufs=1))
        hpool = ctx.enter_context(tc.tile_pool(name="hp", bufs=3))
        w0pool = ctx.enter_context(tc.tile_pool(name="w0p", bufs=3))
        recp = ctx.enter_context(tc.tile_pool(name="recp", bufs=2))
        stp = ctx.enter_context(tc.tile_pool(name="stp", bufs=4))
        smalls = ctx.enter_context(tc.tile_pool(name="smalls", bufs=2))
        pa = ctx.enter_context(tc.tile_pool(name="pa", bufs=2, space="PSUM"))
        pd = ctx.enter_context(tc.tile_pool(name="pd", bufs=2, space="PSUM"))
        pn = ctx.enter_context(tc.tile_pool(name="pn", bufs=2, space="PSUM"))
        pp = ctx.enter_context(tc.tile_pool(name="pp", bufs=2, space="PSUM"))
        dram = ctx.enter_context(tc.tile_pool(name="dram", bufs=2, space="DRAM"))

        # ---- params into SBUF ----
        stream = sb.tile([P, R], BF16, tag="stream")
        riW0 = sb.tile([C_IN, H], BF16, tag="riW0")
        riW1 = sb.tile([H, H], BF16, tag="riW1")
        dW0 = [sb.tile([H, H], BF16, tag=f"dW0_{l}", name=f"dW0_{l}")
               for l in range(L)]
        dW1w = [sb.tile([H, H], BF16, tag=f"dW1w_{l}", name=f"dW1w_{l}")
                for l in range(L)]
        dW1p = [sb.tile([H, D], BF16, tag=f"dW1p_{l}", name=f"dW1p_{l}")
                for l in range(L)]
        roW0 = sb.tile([H, H], BF16, tag="roW0")
        roW1 = sb.tile([H, C_OUT], BF16, tag="roW1")
        gT = sb.tile([H, 6], F32, tag="gT")
        beT = sb.tile([H, 6], F32, tag="beT")
        s1t1 = sb.tile([H, 2], F32, tag="s1t1")
        epst = sb.tile([H, 1], F32, tag="epst")

        nc.sync.dma_start(out=riW0, in_=riW0_d.ap())
        nc.sync.dma_start(out=riW1, in_=riW1_d.ap())
        for l in range(L):
            nc.sync.dma_start(out=dW0[l], in_=dW0_d.ap()[l])
            nc.sync.dma_start(out=dW1w[l], in_=dW1w_d.ap()[l])
            nc.sync.dma_start(out=dW1p[l], in_=dW1p_d.ap()[l])
        nc.sync.dma_start(out=roW0, in_=roW0_d.ap())
        nc.sync.dma_start(out=roW1, in_=roW1_d.ap())
        nc.sync.dma_start(out=gT, in_=g_d.ap())
        nc.sync.dma_start(out=beT, in_=be_d.ap())
        nc.sync.dma_start(out=s1t1, in_=s1t1_d.ap())
        nc.vector.memset(epst, EPS)

        def stats_merge(rec, k):
            """rec [P, T, 6] local bn_stats records -> (s, t) for BN layer k+2."""
            mv = smalls.tile([P, 2], F32, tag="mv")
            nc.vector.bn_aggr(out=mv, in_=rec[:])
            rec3 = smalls.tile([P, 3], F32, tag="rec3")
            nc.vector.memset(rec3[:, 0:1], float(R))
            nc.vector.tensor_copy(out=rec3[:, 1:2], in_=mv[:, 0:1])
            nc.vector.tensor_scalar_mul(out=rec3[:, 2:3], in0=mv[:, 1:2],
                                        scalar1=float(R))
            cc_in = dram.tile([P, 3], F32, tag="cc_in")
            cc_out = dram.tile([NCORES * P, 3], F32, tag="cc_out")
            nc.sync.dma_start(out=cc_in[:], in_=rec3[:])
            nc.gpsimd.collective_compute(
                "AllGather", mybir.AluOpType.bypass,
                replica_groups=[list(range(NCORES))],
                ins=[cc_in.opt()], outs=[cc_out.opt()],
            )
            gath = smalls.tile([P, NCORES, 3], F32, tag="gath")
            src = bass.AP(tensor=cc_out.tensor, offset=cc_out.offset,
                          ap=[[3, P], [P * 3, NCORES], [1, 3]])
            nc.sync.dma_start(out=gath[:], in_=src)
            gmv = smalls.tile([P, 2], F32, tag="gmv")
            nc.vector.bn_aggr(out=gmv, in_=gath[:])
            s = stp.tile([P, 1], F32, tag="s")
            t = stp.tile([P, 1], F32, tag="t")
            nc.scalar.activation(out=s, in_=gmv[:, 1:2],
                                 func=mybir.ActivationFunctionType.Sqrt,
                                 bias=epst[:], scale=1.0)
            nc.vector.reciprocal(out=s, in_=s)
            nc.vector.tensor_mul(out=s, in0=s, in1=gT[:, k:k + 1])
            nc.vector.tensor_mul(out=t, in0=gmv[:, 0:1], in1=s)
            nc.vector.tensor_sub(out=t, in0=beT[:, k:k + 1], in1=t)
            return s, t

        ts = bass.ts

        # ---- PH1: L1 (host stats) -> x1 -> a2 + stats ----
        rec = recp.tile([P, T, 6], F32, tag="rec")
        for i in range(T):
            w0 = w0pool.tile([C_IN, TF], BF16, tag="w0")
            nc.sync.dma_start(out=w0, in_=w0t_d.ap()[:, ts(i, TF)])
            a1 = pa.tile([P, TF], F32, tag="a1")
            nc.tensor.matmul(out=a1[:], lhsT=riW0[:], rhs=w0[:],
                             start=True, stop=True)
            nc.scalar.activation(out=stream[:, ts(i, TF)], in_=a1[:],
                                 func=mybir.ActivationFunctionType.Lrelu,
                                 bias=s1t1[:, 1:2], scale=s1t1[:, 0:1],
                                 alpha=SLOPE)
            a2 = pn.tile([P, TF], F32, tag="a2")
            nc.tensor.matmul(out=a2[:], lhsT=riW1[:], rhs=stream[:, ts(i, TF)],
                             start=True, stop=True)
            nc.vector.bn_stats(out=rec[:, i, :], in_=a2[:])
        s, t = stats_merge(rec, 0)

        # ---- PH2: L2 recompute -> w -> a3 + stats ----
        rec = recp.tile([P, T, 6], F32, tag="rec")
        for i in range(T):
            a2 = pa.tile([P, TF], F32, tag="a1")
            nc.tensor.matmul(out=a2[:], lhsT=riW1[:], rhs=stream[:, ts(i, TF)],
                             start=True, stop=True)
            nc.scalar.activation(out=stream[:, ts(i, TF)], in_=a2[:],
                                 func=mybir.ActivationFunctionType.Lrelu,
                                 bias=t[:], scale=s[:], alpha=SLOPE)
            a3 = pn.tile([P, TF], F32, tag="a2")
            nc.tensor.matmul(out=a3[:], lhsT=dW0[0][:],
                             rhs=stream[:, ts(i, TF)], start=True, stop=True)
            nc.vector.bn_stats(out=rec[:, i, :], in_=a3[:])
        s, t = stats_merge(rec, 1)

        # ---- PH3..PH6: blocks ----
        for l in range(L):
            rec = recp.tile([P, T, 6], F32, tag="rec")
            nxt = dW0[l + 1] if l + 1 < L else roW0
            for i in range(T):
                a = pa.tile([P, TF], F32, tag="a1")
                nc.tensor.matmul(out=a[:], lhsT=dW0[l][:],
                                 rhs=stream[:, ts(i, TF)], start=True, stop=True)
                h = hpool.tile([P, TF], BF16, tag="h")
                nc.scalar.activation(out=h, in_=a[:],
                                     func=mybir.ActivationFunctionType.Lrelu,
                                     bias=t[:], scale=s[:], alpha=SLOPE)
                dw = pd.tile([P, TF], F32, tag="dw")
                nc.tensor.matmul(out=dw[:], lhsT=dW1w[l][:], rhs=h[:],
                                 start=True, stop=True)
                dp = pp.tile([D, TF], F32, tag="dp")
                nc.tensor.matmul(out=dp[:], lhsT=dW1p[l][:], rhs=h[:],
                                 start=True, stop=True)
                dpsb = hpool.tile([D, TF], BF16, tag="dpsb")
                nc.scalar.copy(out=dpsb, in_=dp[:])
                nc.sync.dma_start(out=dpos_d.ap()[l, :, ts(i, TF)], in_=dpsb[:])
                nc.vector.tensor_add(out=stream[:, ts(i, TF)],
                                     in0=stream[:, ts(i, TF)], in1=dw[:])
                an = pn.tile([P, TF], F32, tag="a2")
                nc.tensor.matmul(out=an[:], lhsT=nxt[:],
                                 rhs=stream[:, ts(i, TF)], start=True, stop=True)
                nc.vector.bn_stats(out=rec[:, i, :], in_=an[:])
            s, t = stats_merge(rec, 2 + l)

        # ---- PH7: readout ----
        for i in range(T):
            a = pa.tile([P, TF], F32, tag="a1")
            nc.tensor.matmul(out=a[:], lhsT=roW0[:], rhs=stream[:, ts(i, TF)],
                             start=True, stop=True)
            h = hpool.tile([P, TF], BF16, tag="h")
            nc.scalar.activation(out=h, in_=a[:],
                                 func=mybir.ActivationFunctionType.Lrelu,
                                 bias=t[:], scale=s[:], alpha=SLOPE)
            o = pp.tile([C_OUT, TF], F32, tag="dp")
            nc.tensor.matmul(out=o[:], lhsT=roW1[:], rhs=h[:],
                             start=True, stop=True)
            osb = hpool.tile([C_OUT, TF], F32, tag="osb")
            nc.scalar.copy(out=osb, in_=o[:])
            nc.sync.dma_start(out=wout_d.ap()[:, ts(i, TF)], in_=osb[:])

    nc.compile()
    return nc


def kernel(positions, weights, batch,
           ri_W0, ri_b0, ri_g0, ri_be0, ri_W1, ri_b1, ri_g1, ri_be1,
           dW0, db0, dg0, dbe0, dW1, db1,
           ro_W0, ro_b0, ro_g0, ro_be0, ro_W1, ro_b1):
    positions = np.asarray(positions, np.float32)
    weights = np.asarray(weights, np.float32)

    if "nc" not in _cache:
        _cache["nc"] = _build()
    nc = _cache["nc"]

    bf = lambda x: np.asarray(x, np.float32).astype(np.float16)

    # host: exact L1 BN stats from the 2x2 second moment of `weights`
    # (linear bias ri_b0 cancels inside BN)
    w64 = weights.astype(np.float64)
    m1 = w64.mean(0)                       # [2]
    m2 = (w64.T @ w64) / N                 # [2,2]
    # device computes a1 with bf16-rounded inputs; match those moments
    W0r = bf(ri_W0).astype(np.float64)
    mu1 = m1 @ W0r
    e2 = np.einsum("kc,kl,lc->c", W0r, m2, W0r)
    var1 = e2 - mu1 * mu1
    s1 = np.asarray(ri_g0, np.float64) / np.sqrt(var1 + EPS)
    t1 = np.asarray(ri_be0, np.float64) - mu1 * s1
    s1t1 = np.stack([s1, t1], 1).astype(np.float32)   # [128, 2]

    gT = np.stack([ri_g1, dg0[0], dg0[1], dg0[2], dg0[3], ro_g0], 1)
    beT = np.stack([ri_be1, dbe0[0], dbe0[1], dbe0[2], dbe0[3], ro_be0], 1)

    dW1 = np.asarray(dW1, np.float32)
    shared = dict(
        riW0=bf(ri_W0), riW1=bf(ri_W1),
        dW0=bf(dW0), dW1w=bf(np.ascontiguousarray(dW1[:, :, D:])),
        dW1p=bf(np.ascontiguousarray(dW1[:, :, :D])),
        roW0=bf(ro_W0), roW1=bf(ro_W1),
        gT=np.asarray(gT, np.float32), beT=np.asarray(beT, np.float32),
        s1t1=s1t1,
    )
    in_maps = []
    for c in range(NCORES):
        sl = weights[c * R:(c + 1) * R]
        in_maps.append(dict(shared, w0t=bf(np.ascontiguousarray(sl.T))))

    trace = bool(int(os.environ.get("KERNEL_TRACE", "0")))
    kw = {}
    if trace:
        _install_trace_hook()
        kw["tmpdir"] = os.environ.get("KERNEL_TRACE_DIR") or None
    res = run_bass_kernel_spmd(
        nc, in_maps, core_ids=list(range(NCORES)), trace=trace, **kw,
    )
    _cache["last_results"] = res

    # assemble
    pos = positions.astype(np.float64)
    db1 = np.asarray(db1, np.float64)
    wout = np.empty((N, C_OUT), np.float32)
    dsum = np.zeros((N, D), np.float64)
    for c in range(NCORES):
        r = res.results[c]
        dsum[c * R:(c + 1) * R] += r["dpos"].astype(np.float64).sum(0).T
        wout[c * R:(c + 1) * R] = r["wout"].T
    pos = pos + dsum + db1[:, :D].sum(0)
    wout = (wout.astype(np.float64) + np.asarray(ro_b1, np.float64)).astype(np.float32)
    return pos.astype(np.float32), wout

